# revision 69
# baseline (speedup 1.0000x reference)
"""Trainium2 Bass kernel for nn_AdaptiveWaveletBank.

out[b, s, n] = sum_k w_s[k] * signal[b, n - wl_s + k]   (complex w, zero-pad)

Strategy:
  - Data-parallel over batch: 16 rows -> 8 cores x 2 rows.
  - The Morlet-like wavelet w_s decays as exp(-0.5 (k/scale)^2): only the
    first ~6.1*scale taps matter (<1e-8 of peak).  Host truncates.
  - Conv as banded matmuls on the TensorEngine: signal tiled 128-wide on
    partitions (several phase-shifted copies), banded Toeplitz A blocks
    (host-built, fp16) as the moving operand, PSUM fp32 accumulation.
    Scales with few taps use an even/odd half-tile mode (two single
    128-col matmuls sharing one A block); long scales use accumulation
    chains over tile shifts.
  - DVE/ACT copy+cast PSUM->fp16 staging laid out so output DMAs are fully
    contiguous; host reassembles complex64.
"""

import numpy as np

import concourse.bacc as bacc
import concourse.bass as bass
import concourse.mybir as mybir
import concourse.tile as tile
from concourse.bass_utils import run_bass_kernel_spmd

B, L, NSC = 16, 32768, 16
CHUNKS = [(0, 2), (2, 8), (8, 16)]
DUMMIES = 0
LAST_SPLIT = 4
LAST_CHEAP_END = False
SIG_SPLIT = False
NCORES = 8
ROWS = B // NCORES          # rows of the batch per core
NT = L // 128               # 256 signal tiles of 128 samples
PAD = 16                    # leading zero tiles (max tile shift)
NUM_OSC = 6.0
ENV_CUT = 2.2e-3            # truncate wavelet where envelope < this
                            # (t_c=3.5 sigma: tail energy erfc(3.5) adds
                            # ~8.6e-4 rel err vs the 2e-2 budget)

F16 = mybir.dt.float16
F32 = mybir.dt.float32
F8 = mybir.dt.float8e3            # e3m4: 4 mantissa bits, max 15.5
NF8 = 16                          # all scales stored as fp8 e3m4
NF16 = NSC - NF8
PRESCALE = 0.65                   # scales 12-15 exceed 15.5; prescale on
PRESCALE_S0 = 12                  # device, undo on host


def _scales_and_lengths():
    s = np.exp(np.linspace(np.log(1.0), np.log(32.0), NSC))
    lengths = []
    for sc in s:
        wl = min(int(L * 0.5), int(64 * sc))
        wl = max(wl, 8)
        wl = wl if wl % 2 == 0 else wl + 1
        lengths.append(wl)
    return s, lengths


def _wavelets(sc, wl, cf, bw):
    # float32 arithmetic to mirror the jnp reference
    t = np.arange(wl, dtype=np.float32) / (bw * np.float32(max(float(sc), 0.1)))
    env = np.exp(-0.5 * t * t).astype(np.float32)
    ph = (np.float32(2.0 * np.pi / NUM_OSC) * cf * t).astype(np.float32)
    wr = env * np.cos(ph)
    wi = env * np.sin(ph)
    norm = np.max(np.sqrt(wr * wr + wi * wi)) + np.float32(1e-8)
    return (wr / norm).astype(np.float32), (wi / norm).astype(np.float32), env


def _plan(cf, bw, grans=(64, 32, 8)):
    """Per-scale mode/truncation plan + packed A matrix + phase list.

    eo mode: window base delta (mult of 64/32/8, >= wl, <= wl+64-kcut);
    even half-tile reads sig[128m - delta + j], odd sig[128m - delta+64 + j];
    both share A[j, 2u+c] = w[wl - delta + j - u].
    chain mode: accumulate over 128-tile shifts t with a 0/64 phase pick.
    """
    s_vals, wlens = _scales_and_lengths()
    scales = []
    cols = 0
    phases = [0, 64]            # base phases kept first
    for sc, wl in zip(s_vals, wlens):
        wr, wi, env = _wavelets(sc, wl, cf, bw)
        kcut = int(np.sum(env > ENV_CUT))
        kcut = max(1, min(kcut, wl))
        delta = None
        if kcut <= 64 and wl >= 64:
            for gran in grans:
                d = gran * (-(-wl // gran))
                if d <= wl + 64 - kcut:
                    delta = d
                    break
        if delta is not None:
            sub = []
            for eo in range(2):
                di = delta - 64 * eo
                sg = di % 128
                if sg not in phases:
                    phases.append(sg)
                sub.append((phases.index(sg), di // 128))
            scales.append(dict(wl=wl, wr=wr, wi=wi, kcut=kcut, mode="eo",
                               delta=delta, sub=tuple(sub), col=cols))
            cols += 128
            continue
        best = None
        for ph in (0, 64):
            t_hi = (wl - ph + 127) // 128
            t_lo = -(-(wl - ph - kcut - 126) // 128)
            if t_lo < 0 and ph > 0:
                continue
            t_lo = max(0, t_lo)
            if best is None or t_hi - t_lo < best[1] - best[0]:
                best = (t_lo, t_hi, ph)
        t_lo, t_hi, ph = best
        ts = list(range(t_lo, t_hi + 1))
        # nonzero u-range of each tile-shift block (band is zero outside);
        # consecutive blocks overlap by kcut-1 which also orders them
        # one block is a full-width start=True umbrella (every other block
        # then accumulates into already-written columns); pick the block
        # with the widest native band as umbrella, others stream only
        # their nonzero band
        nat = []
        for t in ts:
            C = wl - ph - 128 * t
            u0 = max(0, min(127, C - kcut + 1))
            u1 = min(127, max(0, C + 127))
            nat.append((u0, u1))
        ui = max(range(len(ts)), key=lambda i: nat[i][1] - nat[i][0])
        ts = [ts[ui]] + ts[:ui] + ts[ui + 1:]
        rng = [(0, 127)] + nat[:ui] + nat[ui + 1:]
        scales.append(dict(wl=wl, wr=wr, wi=wi, kcut=kcut, mode="chain",
                           ts=ts, col=cols, ph=ph, rng=tuple(rng)))
        cols += len(ts) * 256

    amat = np.zeros((128, cols), dtype=np.float16)
    j = np.arange(128)[:, None]
    for sp in scales:
        wl, wr, wi, kcut = sp["wl"], sp["wr"], sp["wi"], sp["kcut"]
        if sp["mode"] == "eo":
            u = np.arange(64)[None, :]
            k = wl - sp["delta"] + j - u
            valid = (k >= 0) & (k < kcut)
            kc = np.clip(k, 0, wl - 1)
            blk = np.zeros((128, 128), dtype=np.float32)
            blk[:, 0::2] = np.where(valid, wr[kc], 0.0)
            blk[:, 1::2] = np.where(valid, wi[kc], 0.0)
            amat[:, sp["col"]:sp["col"] + 128] = blk.astype(np.float16)
            continue
        u = np.arange(128)[None, :]
        for i, t in enumerate(sp["ts"]):
            k = wl - sp["ph"] + j - u - 128 * t
            valid = (k >= 0) & (k < kcut)
            kc = np.clip(k, 0, wl - 1)
            blk = np.zeros((128, 256), dtype=np.float32)
            blk[:, 0::2] = np.where(valid, wr[kc], 0.0)
            blk[:, 1::2] = np.where(valid, wi[kc], 0.0)
            off = sp["col"] + i * 256
            amat[:, off:off + 256] = blk.astype(np.float16)
    return scales, amat, phases


NSTRIP = 3


def _make_sig(sig_rows, phases):
    """(ROWS, L) fp32 -> (128, ROWS, 3, PAD+NT) fp16 tiled/padded.

    Three strips per row so both eo merge patterns are contiguous
    2-strip slices for the swapped (A-stationary) eo matmuls:
      strip0 = phase-64 copy   (x[i] = sig[i-64])
      strip1 = phase-0 copy    (x[i] = sig[i])
      strip2 = strip0 shifted left one tile (tile t = phase64 tile t+1)
    Pattern A (delta%128==64): rhs = strips (0,1); B (==0): strips (1,2).
    Partition-major so the device DMA is one contiguous line/partition."""
    st = np.zeros((ROWS, NSTRIP, 128, PAD + NT), dtype=np.float16)
    s16 = sig_rows.astype(np.float16)
    for r in range(ROWS):
        x0 = s16[r]
        x1 = np.zeros(L, dtype=np.float16)
        x1[64:] = s16[r][:L - 64]
        st[r, 0, :, PAD:] = x1.reshape(NT, 128).T
        st[r, 1, :, PAD:] = x0.reshape(NT, 128).T
        st[r, 2, :, 0:PAD + NT - 1] = st[r, 0, :, 1:]
    return np.ascontiguousarray(st.transpose(2, 0, 1, 3))


def _unit_pairs(grp):
    """Scale pairs per group; group 1 reversed so the kernel tail ends on a
    cheap eo unit."""
    return [(grp * 8 + 2 * i, grp * 8 + 2 * i + 1) for i in range(4)]


def _build_nc(scales, acols, nph):
    """Build + schedule + compile the per-core Bass program."""
    nc = bacc.Bacc("TRN2", target_bir_lowering=False, debug=False,
                   num_devices=NCORES)

    sig_d = nc.dram_tensor("sig", [128, ROWS, NSTRIP, PAD + NT], F16,
                           kind="ExternalInput")
    amat_d = nc.dram_tensor("amat", [128, acols], F16, kind="ExternalInput")
    # out[row, half, c, s, 2u+comp] ; n = half*16384 + c*128 + u
    # all scales fp8 e3m4; scales >= PRESCALE_S0 written as v*PRESCALE to
    # fit the 15.5 e3m4 max (host multiplies back)
    out8_d = nc.dram_tensor("out8", [ROWS, 2, 128, NSC, 256], F8,
                            kind="ExternalOutput")

    with tile.TileContext(nc) as tc:
        with tc.tile_pool(name="const", bufs=1) as const_pool, \
             tc.tile_pool(name="ob", bufs=16) as ob_pool, \
             tc.tile_pool(name="ps", bufs=1, space="PSUM") as ps_pool:

            wz = const_pool.tile([128, 512], F16, tag="wz")
            wz2 = const_pool.tile([128, 8], F16, tag="wz2")
            # gpsimd boots ~5.9us — earliest engine that can seed wz, so
            # warm-up matmuls can start at the Tensor preamble end (~7.6us)
            # instead of waiting for the first input DMA (~9.5us)
            nc.gpsimd.memset(wz[:], 0)

            amat_t = const_pool.tile([128, acols], F16, tag="amat")
            sig_all = const_pool.tile([128, NSTRIP * ROWS * (PAD + NT)],
                                      F16, tag="sig")

            def acol(s):
                return scales[s]["col"] if s < NSC else acols

            def amat_dma(s0, s1, eng=None):
                c0, c1 = acol(s0), acol(s1)
                (eng or nc.scalar).dma_start(out=amat_t[:, c0:c1],
                                             in_=amat_d.ap()[:, c0:c1])

            def sig_dma(r, eng):
                # sig_d is host-transposed to [128, ROWS, 3, PAD+NT]:
                # contiguous per-partition lines; per-row DMAs on separate
                # trigger queues overlap their read latencies (m-splits
                # regress: strided descriptors halve DMA efficiency)
                w = NSTRIP * (PAD + NT)
                eng.dma_start(
                    out=sig_all[:, r * w:(r + 1) * w],
                    in_=sig_d.ap()[:, r]
                        .rearrange("j p m -> j (p m)"))

            def sig_dma_p(r, p0, p1, eng):
                # partition-range half of one row: halves the serial
                # read-descriptor stream (~250ns/descriptor, 8/queue)
                w = NSTRIP * (PAD + NT)
                eng.dma_start(
                    out=sig_all[p0:p1, r * w:(r + 1) * w],
                    in_=sig_d.ap()[p0:p1, r]
                        .rearrange("j p m -> j (p m)"))

            # row-0 sig halves split across both trigger queues + amat
            # chunk 0: the first matmul needs only these three, and each
            # dma_start carries ~2us completion latency (gpsimd SWDGE as a
            # third ring regresses badly — ~8us slower)
            sig_dma_p(0, 0, 64, nc.scalar)
            sig_dma_p(0, 64, 128, nc.sync)
            amat_dma(*CHUNKS[0], eng=nc.sync)
            sig_dma(1, nc.sync)
            for c0, c1 in CHUNKS[1:]:
                amat_dma(c0, c1)

            # ACT warm-up (activation table load ~1.5-2.7us) sourced from
            # the first amat chunk: no gpsimd/memset dependency
            nc.scalar.copy(wz2[:], amat_t[:, 0:8])

            def sig_slice(r, p, lo, hi):
                # phase p=1 (shift 64) lives in strip 0, p=0 in strip 1
                base = (r * NSTRIP + (1 - p)) * (PAD + NT)
                return sig_all[:, base + lo:base + hi]

            def sig_pair(r, strip, lo):
                # [j, 2 strips, 128 tiles] moving operand for merged eo
                sv = sig_all[:].rearrange("j (r s m) -> j r s m",
                                          r=ROWS, s=NSTRIP)
                return sv[:, r, strip:strip + 2, lo:lo + 128]

            # PE warm-up: dummy matmuls sourced from amat chunk 0 start the
            # DVFS clock ramp during the input DMAs (without sustained PE
            # activity the clock never reaches 2.4 GHz)
            if DUMMIES:
                # continuous PE activity from ~7.6us until the sig gate
                # (~12.2us) so the DVFS clock is at 2.4 GHz when the real
                # span starts (cold matmuls run 2x slower)
                dmy = ps_pool.tile([128, 2, 512], F32, tag="ps0")
                for _ in range(DUMMIES):
                    nc.tensor.matmul(dmy[:, 0, :], wz[:, 0:128], wz[:],
                                     start=True, stop=True)

            def emit_matmuls(ps, j, s, row, half):
                sp = scales[s]
                if sp["mode"] == "eo":
                    # swapped operands: A block stationary (ONE weight
                    # load), both eo windows merged into a 2-strip moving
                    # operand (256 cols) — 128-col matmuls can't hide the
                    # next LDWEIGHTS and run at ~2x cost otherwise.
                    # psum comes out transposed: [2u+comp, eo*128 + c]
                    p0, q0 = sp["sub"][0]
                    strip = 0 if p0 == 1 else 1
                    lo = PAD + 128 * half - q0
                    nc.tensor.matmul(
                        ps[:, j, 0:256],
                        amat_t[:, sp["col"]:sp["col"] + 128],
                        sig_pair(row, strip, lo),
                        start=True, stop=True,
                    )
                    return
                nts = len(sp["ts"])
                for i, t in enumerate(sp["ts"]):
                    lo = PAD + 128 * half - t
                    u0, u1 = sp["rng"][i]
                    c0 = sp["col"] + i * 256 + 2 * u0
                    c1 = sp["col"] + i * 256 + 2 * u1 + 2
                    nc.tensor.matmul(
                        ps[:, j, 2 * u0:2 * u1 + 2],
                        sig_slice(row, sp["ph"] // 64, lo, lo + 128),
                        amat_t[:, c0:c1],
                        start=(i == 0),
                        stop=(i == nts - 1),
                    )

            pg = 0

            def emit_pair(grp, row, half, pair, sA, sB, ob8, last_rh):
                nonlocal pg
                pg += 1
                ps = ps_pool.tile([128, 2, 512], F32, tag=f"ps{pg % 4}")
                emit_matmuls(ps, 0, sA, row, half)
                emit_matmuls(ps, 1, sB, row, half)
                d0 = sA % 8
                dst = ob8[:, d0:d0 + 2, :]
                src2 = ps[:, :, 0:256]
                # gpsimd cannot access PSUM; ACT+DVE only.
                # (giving ACT a 3rd pair-copy on some units regresses
                # ~2us: local psum-rotation stall)
                # scales >= PRESCALE_S0 shrink into e3m4 range
                on_act = (pair % 2 == 0) if last_rh else (pair in (0, 3))
                if sA >= PRESCALE_S0:
                    if on_act:
                        nc.scalar.mul(dst, src2, PRESCALE)
                    else:
                        nc.vector.tensor_scalar_mul(dst, src2, PRESCALE)
                elif on_act:
                    nc.scalar.copy(dst, src2)
                else:
                    nc.vector.tensor_copy(dst, src2)

            def emit_dma(grp, row, half, ob8, last_rh):
                if last_rh:
                    # drain: 12-15 (done first) as one DMA, then 8-9 and
                    # 10-11 separately so the kernel's final receipt
                    # covers only 512B/partition
                    segs = ((nc.sync, 4, 8), (nc.scalar, 0, 2),
                            (nc.sync, 2, 4))
                    for eng, a, b in segs:
                        s0q = grp * 8 + a
                        eng.dma_start(
                            out=out8_d.ap()[row, half, :,
                                            s0q:s0q + (b - a), :]
                                .rearrange("c s i -> c (s i)"),
                            in_=ob8[:, a:b, :]
                                .rearrange("c s i -> c (s i)"),
                        )
                else:
                    dma_eng = nc.sync if (row + half) % 2 == 0 \
                        else nc.scalar
                    dma_eng.dma_start(
                        out=out8_d.ap()[row, half, :,
                                        grp * 8:(grp + 1) * 8, :]
                            .rearrange("c s i -> c (s i)"),
                        in_=ob8[:].rearrange("c s i -> c (s i)"),
                    )

            # grp0: sequential units (pair-interleaving the two halves
            # regresses ~1.3us: the h0 output DMA fires too late)
            for row in range(ROWS):
                for half in range(2):
                    ob8 = ob_pool.tile([128, 8, 256], F8, tag="ob80")
                    for pair, (sA, sB) in enumerate(_unit_pairs(0)):
                        emit_pair(0, row, half, pair, sA, sB, ob8, False)
                    emit_dma(0, row, half, ob8, False)

            # grp1: sequential units
            for row in range(ROWS):
                for half in range(2):
                    last_rh = (row == ROWS - 1 and half == 1)
                    ob8 = ob_pool.tile([128, 8, 256], F8, tag="ob81")
                    upairs = _unit_pairs(1)
                    if last_rh:
                        # chain pairs first; their copies+DMA are the
                        # kernel's final drain, so start them earliest
                        upairs = [upairs[2], upairs[3],
                                  upairs[0], upairs[1]]
                    for pair, (sA, sB) in enumerate(upairs):
                        emit_pair(1, row, half, pair, sA, sB, ob8, last_rh)
                    emit_dma(1, row, half, ob8, last_rh)
    nc.compile()
    return nc


_CACHE = {}


def _get_nc(key, scales, acols, nph):
    if key not in _CACHE:
        _CACHE[key] = _build_nc(scales, acols, nph)
    return _CACHE[key]


def _plan_key(scales, phases):
    return tuple((sp["mode"], sp["col"], sp.get("delta", -1),
                  tuple(sp.get("sub", ())), tuple(sp.get("ts", ())),
                  sp.get("ph", -1), tuple(sp.get("rng", ())))
                 for sp in scales) + tuple(phases) \
        + tuple(CHUNKS) + (DUMMIES, LAST_SPLIT, LAST_CHEAP_END,
                            SIG_SPLIT)


GRANS = (64,)


def kernel(signal, scales_log, center_freq_log, bandwidth_log):
    signal = np.asarray(signal, dtype=np.float32)
    cf = np.float32(np.exp(np.float32(np.asarray(center_freq_log))))
    bw = np.float32(np.exp(np.float32(np.asarray(bandwidth_log))))

    scales, amat, phases = _plan(cf, bw, GRANS)
    nc = _get_nc(_plan_key(scales, phases), scales, amat.shape[1],
                 len(phases))

    in_maps = []
    for core in range(NCORES):
        st = _make_sig(signal[core * ROWS:(core + 1) * ROWS], phases)
        in_maps.append({"sig": st, "amat": amat})

    res = run_bass_kernel_spmd(nc, in_maps, core_ids=list(range(NCORES)))

    out = np.empty((B, NSC, L), dtype=np.complex64)
    for core in range(NCORES):
        r0 = core * ROWS
        o = np.asarray(res.results[core]["out8"]).astype(np.float32)
        out[r0:r0 + ROWS] = _assemble(o, scales)
    out[:, PRESCALE_S0:] *= np.float32(1.0 / PRESCALE)
    return out


def _assemble(o, scales):
    """Device blocks [ROWS, 2, 128, NSC, 256] fp32 -> (ROWS, NSC, L)
    complex64 (prescale NOT undone here).

    chain scales: [row, half, c, s, 2u+comp], n = half*16384 + c*128 + u
    eo scales (swapped matmul): [row, half, 2u+comp, s, eo*128 + c],
      n = half*16384 + c*128 + 64*eo + u"""
    res = np.empty((ROWS, NSC, L), dtype=np.complex64)
    for s, sp in enumerate(scales):
        blk = o[:, :, :, s, :]                         # [row, half, P, C]
        if sp["mode"] == "eo":
            b = blk.reshape(ROWS, 2, 64, 2, 2, 128)    # r,h,u,comp,e,c
            b = b.transpose(0, 1, 5, 4, 2, 3)          # r,h,c,e,u,comp
            b = np.ascontiguousarray(b).reshape(ROWS, L, 2)
        else:
            b = blk.reshape(ROWS, 2, 128, 128, 2)      # r,h,c,u,comp
            b = b.reshape(ROWS, L, 2)
        res[:, s] = b[..., 0] + 1j * b[..., 1]
    return res



# revision 70
# speedup vs baseline: 1.1712x; 1.1712x over previous
"""Trainium2 Bass kernel for nn_AdaptiveWaveletBank.

out[b, s, n] = sum_k w_s[k] * signal[b, n - wl_s + k]   (complex w, zero-pad)

Strategy:
  - Data-parallel over batch: 16 rows -> 8 cores x 2 rows.
  - The Morlet-like wavelet w_s decays as exp(-0.5 (k/scale)^2): only the
    first ~6.1*scale taps matter (<1e-8 of peak).  Host truncates.
  - Conv as banded matmuls on the TensorEngine: signal tiled 128-wide on
    partitions (several phase-shifted copies), banded Toeplitz A blocks
    (host-built, fp16) as the moving operand, PSUM fp32 accumulation.
    Scales with few taps use an even/odd half-tile mode (two single
    128-col matmuls sharing one A block); long scales use accumulation
    chains over tile shifts.
  - DVE/ACT copy+cast PSUM->fp16 staging laid out so output DMAs are fully
    contiguous; host reassembles complex64.
"""

import numpy as np

import concourse.bacc as bacc
import concourse.bass as bass
import concourse.mybir as mybir
import concourse.tile as tile
from concourse.bass_utils import run_bass_kernel_spmd

B, L, NSC = 16, 32768, 16
CHUNKS = [(0, 2), (2, 8), (8, 16)]
DUMMIES = 8
LAST_SPLIT = 4
LAST_CHEAP_END = False
SIG_SPLIT = False
NCORES = 8
ROWS = B // NCORES          # rows of the batch per core
NT = L // 128               # 256 signal tiles of 128 samples
PAD = 16                    # leading zero tiles (max tile shift)
NUM_OSC = 6.0
ENV_CUT = 2.2e-3            # truncate wavelet where envelope < this
                            # (t_c=3.5 sigma: tail energy erfc(3.5) adds
                            # ~8.6e-4 rel err vs the 2e-2 budget)

F16 = mybir.dt.float16
F32 = mybir.dt.float32
F8 = mybir.dt.float8e3            # e3m4: 4 mantissa bits, max 15.5
NF8 = 16                          # all scales stored as fp8 e3m4
NF16 = NSC - NF8
PRESCALE = 0.65                   # scales 12-15 exceed 15.5; prescale on
PRESCALE_S0 = 12                  # device, undo on host


def _scales_and_lengths():
    s = np.exp(np.linspace(np.log(1.0), np.log(32.0), NSC))
    lengths = []
    for sc in s:
        wl = min(int(L * 0.5), int(64 * sc))
        wl = max(wl, 8)
        wl = wl if wl % 2 == 0 else wl + 1
        lengths.append(wl)
    return s, lengths


def _wavelets(sc, wl, cf, bw):
    # float32 arithmetic to mirror the jnp reference
    t = np.arange(wl, dtype=np.float32) / (bw * np.float32(max(float(sc), 0.1)))
    env = np.exp(-0.5 * t * t).astype(np.float32)
    ph = (np.float32(2.0 * np.pi / NUM_OSC) * cf * t).astype(np.float32)
    wr = env * np.cos(ph)
    wi = env * np.sin(ph)
    norm = np.max(np.sqrt(wr * wr + wi * wi)) + np.float32(1e-8)
    return (wr / norm).astype(np.float32), (wi / norm).astype(np.float32), env


def _plan(cf, bw, grans=(64, 32, 8)):
    """Per-scale mode/truncation plan + packed A matrix + phase list.

    eo mode: window base delta (mult of 64/32/8, >= wl, <= wl+64-kcut);
    even half-tile reads sig[128m - delta + j], odd sig[128m - delta+64 + j];
    both share A[j, 2u+c] = w[wl - delta + j - u].
    chain mode: accumulate over 128-tile shifts t with a 0/64 phase pick.
    """
    s_vals, wlens = _scales_and_lengths()
    scales = []
    cols = 0
    phases = [0, 64]            # base phases kept first
    for sc, wl in zip(s_vals, wlens):
        wr, wi, env = _wavelets(sc, wl, cf, bw)
        kcut = int(np.sum(env > ENV_CUT))
        kcut = max(1, min(kcut, wl))
        delta = None
        if kcut <= 64 and wl >= 64:
            for gran in grans:
                d = gran * (-(-wl // gran))
                if d <= wl + 64 - kcut:
                    delta = d
                    break
        if delta is not None:
            sub = []
            for eo in range(2):
                di = delta - 64 * eo
                sg = di % 128
                if sg not in phases:
                    phases.append(sg)
                sub.append((phases.index(sg), di // 128))
            scales.append(dict(wl=wl, wr=wr, wi=wi, kcut=kcut, mode="eo",
                               delta=delta, sub=tuple(sub), col=cols))
            cols += 128
            continue
        best = None
        for ph in (0, 64):
            t_hi = (wl - ph + 127) // 128
            t_lo = -(-(wl - ph - kcut - 126) // 128)
            if t_lo < 0 and ph > 0:
                continue
            t_lo = max(0, t_lo)
            if best is None or t_hi - t_lo < best[1] - best[0]:
                best = (t_lo, t_hi, ph)
        t_lo, t_hi, ph = best
        ts = list(range(t_lo, t_hi + 1))
        # nonzero u-range of each tile-shift block (band is zero outside);
        # consecutive blocks overlap by kcut-1 which also orders them
        # one block is a full-width start=True umbrella (every other block
        # then accumulates into already-written columns); pick the block
        # with the widest native band as umbrella, others stream only
        # their nonzero band
        nat = []
        for t in ts:
            C = wl - ph - 128 * t
            u0 = max(0, min(127, C - kcut + 1))
            u1 = min(127, max(0, C + 127))
            nat.append((u0, u1))
        ui = max(range(len(ts)), key=lambda i: nat[i][1] - nat[i][0])
        ts = [ts[ui]] + ts[:ui] + ts[ui + 1:]
        rng = [(0, 127)] + nat[:ui] + nat[ui + 1:]
        scales.append(dict(wl=wl, wr=wr, wi=wi, kcut=kcut, mode="chain",
                           ts=ts, col=cols, ph=ph, rng=tuple(rng)))
        cols += len(ts) * 256

    amat = np.zeros((128, cols), dtype=np.float16)
    j = np.arange(128)[:, None]
    for sp in scales:
        wl, wr, wi, kcut = sp["wl"], sp["wr"], sp["wi"], sp["kcut"]
        if sp["mode"] == "eo":
            u = np.arange(64)[None, :]
            k = wl - sp["delta"] + j - u
            valid = (k >= 0) & (k < kcut)
            kc = np.clip(k, 0, wl - 1)
            blk = np.zeros((128, 128), dtype=np.float32)
            blk[:, 0::2] = np.where(valid, wr[kc], 0.0)
            blk[:, 1::2] = np.where(valid, wi[kc], 0.0)
            amat[:, sp["col"]:sp["col"] + 128] = blk.astype(np.float16)
            continue
        u = np.arange(128)[None, :]
        for i, t in enumerate(sp["ts"]):
            k = wl - sp["ph"] + j - u - 128 * t
            valid = (k >= 0) & (k < kcut)
            kc = np.clip(k, 0, wl - 1)
            blk = np.zeros((128, 256), dtype=np.float32)
            blk[:, 0::2] = np.where(valid, wr[kc], 0.0)
            blk[:, 1::2] = np.where(valid, wi[kc], 0.0)
            off = sp["col"] + i * 256
            amat[:, off:off + 256] = blk.astype(np.float16)
    return scales, amat, phases


NSTRIP = 3


def _make_sig(sig_rows, phases):
    """(ROWS, L) fp32 -> (128, ROWS, 3, PAD+NT) fp16 tiled/padded.

    Three strips per row so both eo merge patterns are contiguous
    2-strip slices for the swapped (A-stationary) eo matmuls:
      strip0 = phase-64 copy   (x[i] = sig[i-64])
      strip1 = phase-0 copy    (x[i] = sig[i])
      strip2 = strip0 shifted left one tile (tile t = phase64 tile t+1)
    Pattern A (delta%128==64): rhs = strips (0,1); B (==0): strips (1,2).
    Partition-major so the device DMA is one contiguous line/partition."""
    st = np.zeros((ROWS, NSTRIP, 128, PAD + NT), dtype=np.float16)
    s16 = sig_rows.astype(np.float16)
    for r in range(ROWS):
        x0 = s16[r]
        x1 = np.zeros(L, dtype=np.float16)
        x1[64:] = s16[r][:L - 64]
        st[r, 0, :, PAD:] = x1.reshape(NT, 128).T
        st[r, 1, :, PAD:] = x0.reshape(NT, 128).T
        st[r, 2, :, 0:PAD + NT - 1] = st[r, 0, :, 1:]
    return np.ascontiguousarray(st.transpose(2, 0, 1, 3))


def _unit_pairs(grp):
    """Scale pairs per group; group 1 reversed so the kernel tail ends on a
    cheap eo unit."""
    return [(grp * 8 + 2 * i, grp * 8 + 2 * i + 1) for i in range(4)]


def _build_nc(scales, acols, nph):
    """Build + schedule + compile the per-core Bass program."""
    nc = bacc.Bacc("TRN2", target_bir_lowering=False, debug=False,
                   num_devices=NCORES)

    sig_d = nc.dram_tensor("sig", [128, ROWS, NSTRIP, PAD + NT], F16,
                           kind="ExternalInput")
    amat_d = nc.dram_tensor("amat", [128, acols], F16, kind="ExternalInput")
    # out[row, half, c, s, 2u+comp] ; n = half*16384 + c*128 + u
    # all scales fp8 e3m4; scales >= PRESCALE_S0 written as v*PRESCALE to
    # fit the 15.5 e3m4 max (host multiplies back)
    out8_d = nc.dram_tensor("out8", [ROWS, 2, 128, NSC, 256], F8,
                            kind="ExternalOutput")

    with tile.TileContext(nc) as tc:
        with tc.tile_pool(name="const", bufs=1) as const_pool, \
             tc.tile_pool(name="ob", bufs=16) as ob_pool, \
             tc.tile_pool(name="ps", bufs=1, space="PSUM") as ps_pool:

            wz = const_pool.tile([128, 512], F16, tag="wz")
            wz2 = const_pool.tile([128, 8], F16, tag="wz2")
            # gpsimd boots ~5.9us — earliest engine that can seed wz, so
            # warm-up matmuls can start at the Tensor preamble end (~7.6us)
            # instead of waiting for the first input DMA (~9.5us)
            nc.gpsimd.memset(wz[:], 0)

            amat_t = const_pool.tile([128, acols], F16, tag="amat")
            sig_all = const_pool.tile([128, NSTRIP * ROWS * (PAD + NT)],
                                      F16, tag="sig")

            def acol(s):
                return scales[s]["col"] if s < NSC else acols

            def amat_dma(s0, s1, eng=None):
                c0, c1 = acol(s0), acol(s1)
                (eng or nc.scalar).dma_start(out=amat_t[:, c0:c1],
                                             in_=amat_d.ap()[:, c0:c1])

            def sig_dma(r, eng):
                # sig_d is host-transposed to [128, ROWS, 3, PAD+NT]:
                # contiguous per-partition lines; per-row DMAs on separate
                # trigger queues overlap their read latencies (m-splits
                # regress: strided descriptors halve DMA efficiency)
                w = NSTRIP * (PAD + NT)
                eng.dma_start(
                    out=sig_all[:, r * w:(r + 1) * w],
                    in_=sig_d.ap()[:, r]
                        .rearrange("j p m -> j (p m)"))

            def sig_dma_p(r, p0, p1, eng):
                # partition-range half of one row: halves the serial
                # read-descriptor stream (~250ns/descriptor, 8/queue)
                w = NSTRIP * (PAD + NT)
                eng.dma_start(
                    out=sig_all[p0:p1, r * w:(r + 1) * w],
                    in_=sig_d.ap()[p0:p1, r]
                        .rearrange("j p m -> j (p m)"))

            # row-0 sig halves split across both trigger queues + amat
            # chunk 0: the first matmul needs only these three, and each
            # dma_start carries ~2us completion latency (gpsimd SWDGE as a
            # third ring regresses badly — ~8us slower)
            sig_dma_p(0, 0, 64, nc.scalar)
            sig_dma_p(0, 64, 128, nc.sync)
            amat_dma(*CHUNKS[0], eng=nc.sync)
            sig_dma(1, nc.sync)
            for c0, c1 in CHUNKS[1:]:
                amat_dma(c0, c1)

            # ACT warm-up (activation table load ~1.5-2.7us) sourced from
            # the first amat chunk: no gpsimd/memset dependency
            nc.scalar.copy(wz2[:], amat_t[:, 0:8])

            def sig_slice(r, p, lo, hi):
                # phase p=1 (shift 64) lives in strip 0, p=0 in strip 1
                base = (r * NSTRIP + (1 - p)) * (PAD + NT)
                return sig_all[:, base + lo:base + hi]

            def sig_pair(r, strip, lo):
                # [j, 2 strips, 128 tiles] moving operand for merged eo
                sv = sig_all[:].rearrange("j (r s m) -> j r s m",
                                          r=ROWS, s=NSTRIP)
                return sv[:, r, strip:strip + 2, lo:lo + 128]

            # PE warm-up: dummy matmuls sourced from amat chunk 0 start the
            # DVFS clock ramp during the input DMAs (without sustained PE
            # activity the clock never reaches 2.4 GHz)
            if DUMMIES:
                # continuous PE activity from ~7.6us until the sig gate
                # (~12.2us) so the DVFS clock is at 2.4 GHz when the real
                # span starts (cold matmuls run 2x slower)
                dmy = ps_pool.tile([128, 2, 512], F32, tag="ps0")
                for _ in range(DUMMIES):
                    nc.tensor.matmul(dmy[:, 0, :], wz[:, 0:128], wz[:],
                                     start=True, stop=True)

            def emit_matmuls(ps, j, s, row, half):
                sp = scales[s]
                if sp["mode"] == "eo":
                    # swapped operands: A block stationary (ONE weight
                    # load), both eo windows merged into a 2-strip moving
                    # operand (256 cols) — 128-col matmuls can't hide the
                    # next LDWEIGHTS and run at ~2x cost otherwise.
                    # psum comes out transposed: [2u+comp, eo*128 + c]
                    p0, q0 = sp["sub"][0]
                    strip = 0 if p0 == 1 else 1
                    lo = PAD + 128 * half - q0
                    nc.tensor.matmul(
                        ps[:, j, 0:256],
                        amat_t[:, sp["col"]:sp["col"] + 128],
                        sig_pair(row, strip, lo),
                        start=True, stop=True,
                    )
                    return
                nts = len(sp["ts"])
                for i, t in enumerate(sp["ts"]):
                    lo = PAD + 128 * half - t
                    u0, u1 = sp["rng"][i]
                    c0 = sp["col"] + i * 256 + 2 * u0
                    c1 = sp["col"] + i * 256 + 2 * u1 + 2
                    nc.tensor.matmul(
                        ps[:, j, 2 * u0:2 * u1 + 2],
                        sig_slice(row, sp["ph"] // 64, lo, lo + 128),
                        amat_t[:, c0:c1],
                        start=(i == 0),
                        stop=(i == nts - 1),
                    )

            pg = 0

            def emit_pair(grp, row, half, pair, sA, sB, ob8, last_rh):
                nonlocal pg
                pg += 1
                ps = ps_pool.tile([128, 2, 512], F32, tag=f"ps{pg % 4}")
                emit_matmuls(ps, 0, sA, row, half)
                emit_matmuls(ps, 1, sB, row, half)
                d0 = sA % 8
                dst = ob8[:, d0:d0 + 2, :]
                src2 = ps[:, :, 0:256]
                # gpsimd cannot access PSUM; ACT+DVE only.
                # (giving ACT a 3rd pair-copy on some units regresses
                # ~2us: local psum-rotation stall)
                # scales >= PRESCALE_S0 shrink into e3m4 range
                on_act = (pair % 2 == 0) if last_rh else (pair in (0, 3))
                if sA >= PRESCALE_S0:
                    if on_act:
                        nc.scalar.mul(dst, src2, PRESCALE)
                    else:
                        nc.vector.tensor_scalar_mul(dst, src2, PRESCALE)
                elif on_act:
                    nc.scalar.copy(dst, src2)
                else:
                    nc.vector.tensor_copy(dst, src2)

            def emit_dma(grp, row, half, ob8, last_rh):
                if last_rh:
                    # drain: 12-15 (done first) as one DMA, then 8-9 and
                    # 10-11 separately so the kernel's final receipt
                    # covers only 512B/partition
                    segs = ((nc.sync, 4, 8), (nc.scalar, 0, 2),
                            (nc.sync, 2, 4))
                    for eng, a, b in segs:
                        s0q = grp * 8 + a
                        eng.dma_start(
                            out=out8_d.ap()[row, half, :,
                                            s0q:s0q + (b - a), :]
                                .rearrange("c s i -> c (s i)"),
                            in_=ob8[:, a:b, :]
                                .rearrange("c s i -> c (s i)"),
                        )
                else:
                    dma_eng = nc.sync if (row + half) % 2 == 0 \
                        else nc.scalar
                    dma_eng.dma_start(
                        out=out8_d.ap()[row, half, :,
                                        grp * 8:(grp + 1) * 8, :]
                            .rearrange("c s i -> c (s i)"),
                        in_=ob8[:].rearrange("c s i -> c (s i)"),
                    )

            # grp0: sequential units (pair-interleaving the two halves
            # regresses ~1.3us: the h0 output DMA fires too late)
            for row in range(ROWS):
                for half in range(2):
                    ob8 = ob_pool.tile([128, 8, 256], F8, tag="ob80")
                    for pair, (sA, sB) in enumerate(_unit_pairs(0)):
                        emit_pair(0, row, half, pair, sA, sB, ob8, False)
                    emit_dma(0, row, half, ob8, False)

            # grp1: sequential units
            for row in range(ROWS):
                for half in range(2):
                    last_rh = (row == ROWS - 1 and half == 1)
                    ob8 = ob_pool.tile([128, 8, 256], F8, tag="ob81")
                    upairs = _unit_pairs(1)
                    if last_rh:
                        # chain pairs first; their copies+DMA are the
                        # kernel's final drain, so start them earliest
                        upairs = [upairs[2], upairs[3],
                                  upairs[0], upairs[1]]
                    for pair, (sA, sB) in enumerate(upairs):
                        emit_pair(1, row, half, pair, sA, sB, ob8, last_rh)
                    emit_dma(1, row, half, ob8, last_rh)
    nc.compile()
    return nc


_CACHE = {}


def _get_nc(key, scales, acols, nph):
    if key not in _CACHE:
        _CACHE[key] = _build_nc(scales, acols, nph)
    return _CACHE[key]


def _plan_key(scales, phases):
    return tuple((sp["mode"], sp["col"], sp.get("delta", -1),
                  tuple(sp.get("sub", ())), tuple(sp.get("ts", ())),
                  sp.get("ph", -1), tuple(sp.get("rng", ())))
                 for sp in scales) + tuple(phases) \
        + tuple(CHUNKS) + (DUMMIES, LAST_SPLIT, LAST_CHEAP_END,
                            SIG_SPLIT)


GRANS = (64,)


def kernel(signal, scales_log, center_freq_log, bandwidth_log):
    signal = np.asarray(signal, dtype=np.float32)
    cf = np.float32(np.exp(np.float32(np.asarray(center_freq_log))))
    bw = np.float32(np.exp(np.float32(np.asarray(bandwidth_log))))

    scales, amat, phases = _plan(cf, bw, GRANS)
    nc = _get_nc(_plan_key(scales, phases), scales, amat.shape[1],
                 len(phases))

    in_maps = []
    for core in range(NCORES):
        st = _make_sig(signal[core * ROWS:(core + 1) * ROWS], phases)
        in_maps.append({"sig": st, "amat": amat})

    res = run_bass_kernel_spmd(nc, in_maps, core_ids=list(range(NCORES)))

    out = np.empty((B, NSC, L), dtype=np.complex64)
    for core in range(NCORES):
        r0 = core * ROWS
        o = np.asarray(res.results[core]["out8"]).astype(np.float32)
        out[r0:r0 + ROWS] = _assemble(o, scales)
    out[:, PRESCALE_S0:] *= np.float32(1.0 / PRESCALE)
    return out


def _assemble(o, scales):
    """Device blocks [ROWS, 2, 128, NSC, 256] fp32 -> (ROWS, NSC, L)
    complex64 (prescale NOT undone here).

    chain scales: [row, half, c, s, 2u+comp], n = half*16384 + c*128 + u
    eo scales (swapped matmul): [row, half, 2u+comp, s, eo*128 + c],
      n = half*16384 + c*128 + 64*eo + u"""
    res = np.empty((ROWS, NSC, L), dtype=np.complex64)
    for s, sp in enumerate(scales):
        blk = o[:, :, :, s, :]                         # [row, half, P, C]
        if sp["mode"] == "eo":
            b = blk.reshape(ROWS, 2, 64, 2, 2, 128)    # r,h,u,comp,e,c
            b = b.transpose(0, 1, 5, 4, 2, 3)          # r,h,c,e,u,comp
            b = np.ascontiguousarray(b).reshape(ROWS, L, 2)
        else:
            b = blk.reshape(ROWS, 2, 128, 128, 2)      # r,h,c,u,comp
            b = b.reshape(ROWS, L, 2)
        res[:, s] = b[..., 0] + 1j * b[..., 1]
    return res

